# revision 34
# baseline (speedup 1.0000x reference)
"""KMeans assignment (vq_codebook) Trainium2 kernel.

argmin_k ||x_b - c_k||^2 for X[65536,1024], C[1024,1024], 8 NeuronCores,
data-parallel over the batch (8192 rows/core), centroids replicated.

Math: argmin_k d2 = argmax_k (X@C^T - ||c||^2/2); row term ||x||^2 dropped.
The cross term runs as a single fp16 matmul pass (fp16 operands are exact in
the PE; accumulation is fp32): X is pre-scaled by 2^12 so small magnitudes
stay in fp16's normal range, C is rounded to fp16 after centering (see
_prep). On the reference inputs the fp16 operand rounding flips 27 / 65536
argmaxes (measured rel err 1.52e-2, under the 2e-2 gate); the fp32-accuracy
hi/lo-split variants cost 1.5-3x more matmul time (fp16+fp8-DoubleRow
correction: 318us; bf16 3-matmul split: 682us) and are not needed at this
tolerance.

The 2^12*||c||^2/2 bias is precomputed on host and broadcast to all
partitions, subtracted on the Vector engine; the row max comes from a
fused tensor_scalar (add 0.0, accum=max — half-rate in the DVE's all-SBUF
2x mode, and the +0.0 identity keeps max_index's equality scan against the
subtracted tensor exact), then max_index yields the argmax. This keeps the
DVE at ~3.05us/b-tile, under the tensor engine's 3.41us, so the matmul
stream sets the pace. (tensor_tensor_reduce would fuse subtract+max in one
op but crashes this execution path's NEFF pipeline, as does
gpsimd.tensor_tensor — both verified empirically.)

DMA layout: one big transfer per X block ([128, 8, 1024] with the 8
f-chunks stacked along the free axis) and two for the C table, keeping the
per-DMA HWDGE overhead (~0.6us each) negligible. Block 0 is split into
column slices (b-tiles 0-1 / 2-3 / 4-7) so the first matmuls start as soon
as ~2.5MB has landed instead of waiting for the full 4MB.
"""
import numpy as np
import concourse.bacc as bacc
import concourse.mybir as mybir
from concourse.tile import TileContext
from concourse.bass_utils import run_bass_kernel_spmd

B, F, K = 65536, 1024, 1024
NCORES = 8
BL = B // NCORES          # rows per core
P = 128
FCH = F // P              # 8 contraction chunks
NH = 512                  # psum half (one bank of fp32)
BBLK = 1024               # rows per steady-state X DMA block
NBLK = BL // BBLK
TPB = BBLK // P           # b-tiles per block
S = 4096.0                # 2^12 scale carried by the X side / psum / bias
DT16 = mybir.dt.float16

_NC_CACHE = {}


def _build(bl):
    nblk = bl // BBLK
    nb = bl // P
    nc = bacc.Bacc("TRN2", target_bir_lowering=False)
    xh = nc.dram_tensor("xh", [F, bl], DT16, kind="ExternalInput")
    ch = nc.dram_tensor("ch", [F, K], DT16, kind="ExternalInput")
    c2s = nc.dram_tensor("c2s", [K], mybir.dt.float32, kind="ExternalInput")
    out = nc.dram_tensor("out", [nb, P, 1], mybir.dt.uint32, kind="ExternalOutput")

    xh_r = xh.rearrange("(f p) b -> p f b", p=P)
    ch_r = ch.rearrange("(f p) k -> p f k", p=P)

    with TileContext(nc) as tc:
        with (
            tc.tile_pool(name="cres", bufs=1) as cres,
            tc.tile_pool(name="x0pool", bufs=1) as x0pool,
            tc.tile_pool(name="xpool", bufs=2) as xpool,
            tc.tile_pool(name="work", bufs=3) as work,
            tc.tile_pool(name="psp", bufs=4, space="PSUM") as psp,
        ):
            # PE clock warmup: the tensor engine idles during the prologue
            # DMAs and would otherwise pay the low/mid p-state ramp on the
            # first real b-tile. A burst of matmuls on a zeroed tile brings
            # it to full clock before block 0's data lands (the ramp state
            # survives short idle gaps but resets after ~3us).
            warm = work.tile([P, NH], DT16, tag="warm")
            nc.vector.memset(warm, 0.0)
            # max-probe slots: per tile only [:,0] is rewritten (the fused
            # tensor_scalar max-accum); slots 1-7 stay at this finite 0.0 so
            # max_index always sees valid probe values (their matches land
            # in ix slots 1-7, which are never read).
            mx = cres.tile([P, 8], mybir.dt.float32)
            nc.vector.memset(mx, 0.0)
            wps = psp.tile([P, K], mybir.dt.float32, tag="ps")
            # 12 bursts: enough to end before block-0's data lands (a
            # longer burst delays real work at cold-clock rates) while
            # keeping the post-warmup idle gap well under the ~3us ramp
            # reset threshold (2 bursts -> 4.8us idle -> reset -> +6.7us).
            for i in range(12):
                nc.tensor.matmul(wps[:, 0:NH], warm[:, 0:P], warm,
                                 start=(i == 0), stop=(i == 11))

            # Prologue order == first-output critical path. The C table
            # gates every b-tile, so it leads, split in two so the second
            # half streams while block-0's first slice loads.
            ch_a = cres.tile([P, FCH // 2, K], DT16)
            ch_b = cres.tile([P, FCH // 2, K], DT16)
            nc.sync.dma_start(ch_a, ch_r[:, 0:FCH // 2, :])
            x0_s = x0pool.tile([P, FCH, 2 * P], DT16)       # b-tiles 0-1
            nc.sync.dma_start(x0_s, xh_r[:, :, 0:2 * P])
            nc.sync.dma_start(ch_b, ch_r[:, FCH // 2:FCH, :])

            # bias broadcast: c2s already holds 2^12 * ||c||^2 / 2. Issued
            # before the remaining block-0 slices: the first DVE subtract
            # (and with it the first PSUM-bank recycle) waits on it.
            c2b = cres.tile([P, K], mybir.dt.float32)
            nc.sync.dma_start(c2b, c2s[None, :].to_broadcast([P, K]))

            x0_c = x0pool.tile([P, FCH, 2 * P], DT16)       # b-tiles 2-3
            nc.sync.dma_start(x0_c, xh_r[:, :, 2 * P:4 * P])

            x0_d = x0pool.tile([P, FCH, 4 * P], DT16)       # b-tiles 4-7
            nc.sync.dma_start(x0_d, xh_r[:, :, 4 * P:BBLK])

            def ch_sl(f, h):
                t = ch_a if f < FCH // 2 else ch_b
                return t[:, f % (FCH // 2), h * NH:(h + 1) * NH]

            def load_blk(blk):
                t = xpool.tile([P, FCH, BBLK], DT16, tag="xblk")
                nc.sync.dma_start(
                    t, xh_r[:, :, blk * BBLK:(blk + 1) * BBLK])
                return t

            blk1 = load_blk(1)

            def epilogue(t, ps):
                a_sb = work.tile([P, K], mybir.dt.float32, tag="a")
                nc.vector.tensor_sub(a_sb, ps, c2b)
                # fused copy + row-max: out=a_sb+0.0 (exact identity, so
                # max_index's equality scan against a_sb still matches),
                # accum = row max. ~350ns cheaper than InstMax: the
                # all-SBUF tensor_scalar runs in the DVE 2x mode.
                a2 = work.tile([P, K], mybir.dt.float32, tag="a2")
                nc.vector.tensor_scalar(
                    out=a2, in0=a_sb, scalar1=0.0, scalar2=None,
                    op0=mybir.AluOpType.add, op1=mybir.AluOpType.max,
                    accum_out=mx[:, 0:1])
                ix = work.tile([P, 8], mybir.dt.uint32, tag="ix")
                nc.vector.max_index(ix, mx, a_sb)
                nc.sync.dma_start(out[t], ix[:, 0:1])

            for blk in range(nblk):
                if blk == 1:
                    xb = blk1
                elif blk >= 2:
                    xb = load_blk(blk)
                for i in range(TPB):
                    t = blk * TPB + i
                    if blk == 0:
                        if i < 2:
                            wt, j = x0_s, i
                        elif i < 4:
                            wt, j = x0_c, i - 2
                        else:
                            wt, j = x0_d, i - 4
                    else:
                        wt, j = xb, i
                    ps = psp.tile([P, K], mybir.dt.float32, tag="ps")
                    for f in range(FCH):
                        wh = wt[:, f, j * P:(j + 1) * P]
                        nc.tensor.matmul(ps[:, 0:NH], wh, ch_sl(f, 0),
                                         start=(f == 0), stop=(f == FCH - 1))
                        nc.tensor.matmul(ps[:, NH:K], wh, ch_sl(f, 1),
                                         start=(f == 0), stop=(f == FCH - 1))
                    epilogue(t, ps)
    nc.finalize()
    return nc


def _get_nc(bl):
    if bl not in _NC_CACHE:
        _NC_CACHE[bl] = _build(bl)
    return _NC_CACHE[bl]


def _prep(X, C):
    """Host-side operand prep: fp16 encodings + scaled bias.

    C is encoded centered (C - 0.5): the induced extra score term
    0.5*sum_i(x_i) is constant per row so the argmax is unchanged, while
    halving |C| roughly halves fp16 quantization error (27 vs 35 argmax
    flips on the reference inputs).
    """
    xh16 = (X * np.float32(S)).astype(np.float16)    # fp16(2^12 * X)
    ch16 = (C - np.float32(0.5)).astype(np.float16)
    xh_t = np.ascontiguousarray(xh16.T)
    ch_t = np.ascontiguousarray(ch16.T)
    # with centered scores s'_k = x@(c_k - 0.5), the objective
    # x@c_k - ||c_k||^2/2 equals s'_k - ||c_k||^2/2 plus the row-constant
    # x@0.5 which cannot change the argmax, so the bias is unchanged.
    c2s = (np.float64(S) * 0.5 * np.sum(C.astype(np.float64) ** 2, axis=1)
           ).astype(np.float32)
    return xh_t, ch_t, c2s


def kernel(X, centroids):
    X = np.ascontiguousarray(np.asarray(X, dtype=np.float32))
    C = np.ascontiguousarray(np.asarray(centroids, dtype=np.float32))
    assert X.shape == (B, F) and C.shape == (K, F)

    xh_t, ch_t, c2s = _prep(X, C)

    nc = _get_nc(BL)
    in_maps = []
    for c in range(NCORES):
        sl = slice(c * BL, (c + 1) * BL)
        in_maps.append({
            "xh": np.ascontiguousarray(xh_t[:, sl]),
            "ch": ch_t,
            "c2s": c2s,
        })
    res = run_bass_kernel_spmd(nc, in_maps, core_ids=list(range(NCORES)))
    out = np.concatenate([r["out"].reshape(-1) for r in res.results])
    return out.astype(np.int32)


# revision 37
# speedup vs baseline: 1.0066x; 1.0066x over previous
"""KMeans assignment (vq_codebook) Trainium2 kernel.

argmin_k ||x_b - c_k||^2 for X[65536,1024], C[1024,1024], 8 NeuronCores,
data-parallel over the batch (8192 rows/core), centroids replicated.

Math: argmin_k d2 = argmax_k (X@C^T - ||c||^2/2); row term ||x||^2 dropped.
The cross term runs as a single fp16 matmul pass (fp16 operands are exact in
the PE; accumulation is fp32): X is pre-scaled by 2^12 so small magnitudes
stay in fp16's normal range, C is rounded to fp16 after centering (see
_prep). On the reference inputs the fp16 operand rounding flips 27 / 65536
argmaxes (measured rel err 1.52e-2, under the 2e-2 gate); the fp32-accuracy
hi/lo-split variants cost 1.5-3x more matmul time (fp16+fp8-DoubleRow
correction: 318us; bf16 3-matmul split: 682us) and are not needed at this
tolerance.

The 2^12*||c||^2/2 bias is precomputed on host and broadcast to all
partitions, subtracted on the Vector engine; the row max comes from a
fused tensor_scalar (add 0.0, accum=max — half-rate in the DVE's all-SBUF
2x mode, and the +0.0 identity keeps max_index's equality scan against the
subtracted tensor exact), then max_index yields the argmax. This keeps the
DVE at ~3.05us/b-tile, under the tensor engine's 3.41us, so the matmul
stream sets the pace. (tensor_tensor_reduce would fuse subtract+max in one
op but crashes this execution path's NEFF pipeline, as does
gpsimd.tensor_tensor — both verified empirically.)

DMA layout: one big transfer per X block ([128, 8, 1024] with the 8
f-chunks stacked along the free axis) and two for the C table, keeping the
per-DMA HWDGE overhead (~0.6us each) negligible. Block 0 is split into
column slices (b-tiles 0-1 / 2-3 / 4-7) so the first matmuls start as soon
as ~2.5MB has landed instead of waiting for the full 4MB.
"""
import numpy as np
import concourse.bacc as bacc
import concourse.mybir as mybir
from concourse.tile import TileContext
from concourse.bass_utils import run_bass_kernel_spmd

B, F, K = 65536, 1024, 1024
NCORES = 8
BL = B // NCORES          # rows per core
P = 128
FCH = F // P              # 8 contraction chunks
NH = 512                  # psum half (one bank of fp32)
BBLK = 1024               # rows per steady-state X DMA block
NBLK = BL // BBLK
TPB = BBLK // P           # b-tiles per block
S = 4096.0                # 2^12 scale carried by the X side / psum / bias
DT16 = mybir.dt.float16

_NC_CACHE = {}


def _build(bl):
    nblk = bl // BBLK
    nb = bl // P
    nc = bacc.Bacc("TRN2", target_bir_lowering=False)
    xh = nc.dram_tensor("xh", [F, bl], DT16, kind="ExternalInput")
    ch = nc.dram_tensor("ch", [F, K], DT16, kind="ExternalInput")
    c2s = nc.dram_tensor("c2s", [K], mybir.dt.float32, kind="ExternalInput")
    out = nc.dram_tensor("out", [nb, P, 1], mybir.dt.uint32, kind="ExternalOutput")

    xh_r = xh.rearrange("(f p) b -> p f b", p=P)
    ch_r = ch.rearrange("(f p) k -> p f k", p=P)

    with TileContext(nc) as tc:
        with (
            tc.tile_pool(name="cres", bufs=1) as cres,
            tc.tile_pool(name="x0pool", bufs=1) as x0pool,
            tc.tile_pool(name="xpool", bufs=2) as xpool,
            tc.tile_pool(name="work", bufs=3) as work,
            tc.tile_pool(name="psp", bufs=4, space="PSUM") as psp,
        ):
            # PE clock warmup: the tensor engine idles during the prologue
            # DMAs and would otherwise pay the low/mid p-state ramp on the
            # first real b-tile. A burst of matmuls on a zeroed tile brings
            # it to full clock before block 0's data lands (the ramp state
            # survives short idle gaps but resets after ~3us).
            warm = work.tile([P, NH], DT16, tag="warm")
            nc.vector.memset(warm, 0.0)
            # max-probe slots: per tile only [:,0] is rewritten (the fused
            # tensor_scalar max-accum); slots 1-7 stay at this finite 0.0 so
            # max_index always sees valid probe values (their matches land
            # in ix slots 1-7, which are never read).
            mx = cres.tile([P, 8], mybir.dt.float32)
            nc.vector.memset(mx, 0.0)
            wps = psp.tile([P, K], mybir.dt.float32, tag="ps")
            # 8 bursts: enough to end before block-0's first data lands
            # (a longer burst delays real work at cold-clock rates) while
            # keeping the post-warmup idle gap well under the ~3us ramp
            # reset threshold (2 bursts -> 4.8us idle -> reset -> +6.7us).
            for i in range(8):
                nc.tensor.matmul(wps[:, 0:NH], warm[:, 0:P], warm,
                                 start=(i == 0), stop=(i == 7))

            # Prologue order == first-output critical path. The C table
            # gates every b-tile; it loads in four chunk-pair pieces with
            # block-0's first slice injected after the first piece, so the
            # earliest matmuls (b-tiles 0-1 x chunks 0-1) start at ~5us
            # instead of waiting for half the table.
            ch_p = []
            for cpi in range(4):
                ch_t = cres.tile([P, 2, K], DT16, tag=f"chp{cpi}")
                ch_p.append(ch_t)
            nc.sync.dma_start(ch_p[0], ch_r[:, 0:2, :])
            x0_s = x0pool.tile([P, FCH, 2 * P], DT16)       # b-tiles 0-1
            nc.sync.dma_start(x0_s, xh_r[:, :, 0:2 * P])
            for cpi in range(1, 4):
                nc.sync.dma_start(ch_p[cpi], ch_r[:, 2 * cpi:2 * cpi + 2, :])

            # bias broadcast: c2s already holds 2^12 * ||c||^2 / 2. Issued
            # before the remaining block-0 slices: the first DVE subtract
            # (and with it the first PSUM-bank recycle) waits on it.
            c2b = cres.tile([P, K], mybir.dt.float32)
            nc.sync.dma_start(c2b, c2s[None, :].to_broadcast([P, K]))

            x0_c = x0pool.tile([P, FCH, 2 * P], DT16)       # b-tiles 2-3
            nc.sync.dma_start(x0_c, xh_r[:, :, 2 * P:4 * P])

            x0_d = x0pool.tile([P, FCH, 4 * P], DT16)       # b-tiles 4-7
            nc.sync.dma_start(x0_d, xh_r[:, :, 4 * P:BBLK])

            def ch_sl(f, h):
                return ch_p[f // 2][:, f % 2, h * NH:(h + 1) * NH]

            def load_blk(blk):
                t = xpool.tile([P, FCH, BBLK], DT16, tag="xblk")
                nc.sync.dma_start(
                    t, xh_r[:, :, blk * BBLK:(blk + 1) * BBLK])
                return t

            blk1 = load_blk(1)

            def epilogue(t, ps):
                a_sb = work.tile([P, K], mybir.dt.float32, tag="a")
                nc.vector.tensor_sub(a_sb, ps, c2b)
                # fused copy + row-max: out=a_sb+0.0 (exact identity, so
                # max_index's equality scan against a_sb still matches),
                # accum = row max. ~350ns cheaper than InstMax: the
                # all-SBUF tensor_scalar runs in the DVE 2x mode.
                a2 = work.tile([P, K], mybir.dt.float32, tag="a2")
                nc.vector.tensor_scalar(
                    out=a2, in0=a_sb, scalar1=0.0, scalar2=None,
                    op0=mybir.AluOpType.add, op1=mybir.AluOpType.max,
                    accum_out=mx[:, 0:1])
                ix = work.tile([P, 8], mybir.dt.uint32, tag="ix")
                nc.vector.max_index(ix, mx, a_sb)
                nc.sync.dma_start(out[t], ix[:, 0:1])

            for blk in range(nblk):
                if blk == 1:
                    xb = blk1
                elif blk >= 2:
                    xb = load_blk(blk)
                for i in range(TPB):
                    t = blk * TPB + i
                    if blk == 0:
                        if i < 2:
                            wt, j = x0_s, i
                        elif i < 4:
                            wt, j = x0_c, i - 2
                        else:
                            wt, j = x0_d, i - 4
                    else:
                        wt, j = xb, i
                    ps = psp.tile([P, K], mybir.dt.float32, tag="ps")
                    for f in range(FCH):
                        wh = wt[:, f, j * P:(j + 1) * P]
                        nc.tensor.matmul(ps[:, 0:NH], wh, ch_sl(f, 0),
                                         start=(f == 0), stop=(f == FCH - 1))
                        nc.tensor.matmul(ps[:, NH:K], wh, ch_sl(f, 1),
                                         start=(f == 0), stop=(f == FCH - 1))
                    epilogue(t, ps)
    nc.finalize()
    return nc


def _get_nc(bl):
    if bl not in _NC_CACHE:
        _NC_CACHE[bl] = _build(bl)
    return _NC_CACHE[bl]


def _prep(X, C):
    """Host-side operand prep: fp16 encodings + scaled bias.

    C is encoded centered (C - 0.5): the induced extra score term
    0.5*sum_i(x_i) is constant per row so the argmax is unchanged, while
    halving |C| roughly halves fp16 quantization error (27 vs 35 argmax
    flips on the reference inputs).
    """
    xh16 = (X * np.float32(S)).astype(np.float16)    # fp16(2^12 * X)
    ch16 = (C - np.float32(0.5)).astype(np.float16)
    xh_t = np.ascontiguousarray(xh16.T)
    ch_t = np.ascontiguousarray(ch16.T)
    # with centered scores s'_k = x@(c_k - 0.5), the objective
    # x@c_k - ||c_k||^2/2 equals s'_k - ||c_k||^2/2 plus the row-constant
    # x@0.5 which cannot change the argmax, so the bias is unchanged.
    c2s = (np.float64(S) * 0.5 * np.sum(C.astype(np.float64) ** 2, axis=1)
           ).astype(np.float32)
    return xh_t, ch_t, c2s


def kernel(X, centroids):
    X = np.ascontiguousarray(np.asarray(X, dtype=np.float32))
    C = np.ascontiguousarray(np.asarray(centroids, dtype=np.float32))
    assert X.shape == (B, F) and C.shape == (K, F)

    xh_t, ch_t, c2s = _prep(X, C)

    nc = _get_nc(BL)
    in_maps = []
    for c in range(NCORES):
        sl = slice(c * BL, (c + 1) * BL)
        in_maps.append({
            "xh": np.ascontiguousarray(xh_t[:, sl]),
            "ch": ch_t,
            "c2s": c2s,
        })
    res = run_bass_kernel_spmd(nc, in_maps, core_ids=list(range(NCORES)))
    out = np.concatenate([r["out"].reshape(-1) for r in res.results])
    return out.astype(np.int32)


# revision 38
# speedup vs baseline: 1.0068x; 1.0002x over previous
"""KMeans assignment (vq_codebook) Trainium2 kernel.

argmin_k ||x_b - c_k||^2 for X[65536,1024], C[1024,1024], 8 NeuronCores,
data-parallel over the batch (8192 rows/core), centroids replicated.

Math: argmin_k d2 = argmax_k (X@C^T - ||c||^2/2); row term ||x||^2 dropped.
The cross term runs as a single fp16 matmul pass (fp16 operands are exact in
the PE; accumulation is fp32): X is pre-scaled by 2^12 so small magnitudes
stay in fp16's normal range, C is rounded to fp16 after centering (see
_prep). On the reference inputs the fp16 operand rounding flips 27 / 65536
argmaxes (measured rel err 1.52e-2, under the 2e-2 gate); the fp32-accuracy
hi/lo-split variants cost 1.5-3x more matmul time (fp16+fp8-DoubleRow
correction: 318us; bf16 3-matmul split: 682us) and are not needed at this
tolerance.

The 2^12*||c||^2/2 bias is precomputed on host and broadcast to all
partitions, subtracted on the Vector engine; the row max comes from a
fused tensor_scalar (add 0.0, accum=max — half-rate in the DVE's all-SBUF
2x mode, and the +0.0 identity keeps max_index's equality scan against the
subtracted tensor exact), then max_index yields the argmax. This keeps the
DVE at ~3.05us/b-tile, under the tensor engine's 3.41us, so the matmul
stream sets the pace. (tensor_tensor_reduce would fuse subtract+max in one
op but crashes this execution path's NEFF pipeline, as does
gpsimd.tensor_tensor — both verified empirically.)

DMA layout: one big transfer per X block ([128, 8, 1024] with the 8
f-chunks stacked along the free axis) and two for the C table, keeping the
per-DMA HWDGE overhead (~0.6us each) negligible. Block 0 is split into
column slices (b-tiles 0-1 / 2-3 / 4-7) so the first matmuls start as soon
as ~2.5MB has landed instead of waiting for the full 4MB.
"""
import numpy as np
import concourse.bacc as bacc
import concourse.mybir as mybir
from concourse.tile import TileContext
from concourse.bass_utils import run_bass_kernel_spmd

B, F, K = 65536, 1024, 1024
NCORES = 8
BL = B // NCORES          # rows per core
P = 128
FCH = F // P              # 8 contraction chunks
NH = 512                  # psum half (one bank of fp32)
BBLK = 1024               # rows per steady-state X DMA block
NBLK = BL // BBLK
TPB = BBLK // P           # b-tiles per block
S = 4096.0                # 2^12 scale carried by the X side / psum / bias
DT16 = mybir.dt.float16

_NC_CACHE = {}


def _build(bl):
    nblk = bl // BBLK
    nb = bl // P
    nc = bacc.Bacc("TRN2", target_bir_lowering=False)
    xh = nc.dram_tensor("xh", [F, bl], DT16, kind="ExternalInput")
    ch = nc.dram_tensor("ch", [F, K], DT16, kind="ExternalInput")
    c2s = nc.dram_tensor("c2s", [K], mybir.dt.float32, kind="ExternalInput")
    out = nc.dram_tensor("out", [nb, P, 1], mybir.dt.uint32, kind="ExternalOutput")

    xh_r = xh.rearrange("(f p) b -> p f b", p=P)
    ch_r = ch.rearrange("(f p) k -> p f k", p=P)

    with TileContext(nc) as tc:
        with (
            tc.tile_pool(name="cres", bufs=1) as cres,
            tc.tile_pool(name="x0pool", bufs=1) as x0pool,
            tc.tile_pool(name="xpool", bufs=2) as xpool,
            tc.tile_pool(name="work", bufs=3) as work,
            tc.tile_pool(name="psp", bufs=4, space="PSUM") as psp,
        ):
            # PE clock warmup: the tensor engine idles during the prologue
            # DMAs and would otherwise pay the low/mid p-state ramp on the
            # first real b-tile. A burst of matmuls on a zeroed tile brings
            # it to full clock before block 0's data lands (the ramp state
            # survives short idle gaps but resets after ~3us).
            warm = work.tile([P, NH], DT16, tag="warm")
            nc.vector.memset(warm, 0.0)
            # max-probe slots: per tile only [:,0] is rewritten (the fused
            # tensor_scalar max-accum); slots 1-7 stay at this finite 0.0 so
            # max_index always sees valid probe values (their matches land
            # in ix slots 1-7, which are never read).
            mx = cres.tile([P, 8], mybir.dt.float32)
            nc.vector.memset(mx, 0.0)
            wps = psp.tile([P, K], mybir.dt.float32, tag="ps")
            # 8 bursts: enough to end before block-0's first data lands
            # (a longer burst delays real work at cold-clock rates) while
            # keeping the post-warmup idle gap well under the ~3us ramp
            # reset threshold (2 bursts -> 4.8us idle -> reset -> +6.7us).
            for i in range(8):
                nc.tensor.matmul(wps[:, 0:NH], warm[:, 0:P], warm,
                                 start=(i == 0), stop=(i == 7))

            # Prologue order == first-output critical path. The C table
            # gates every b-tile; it loads in four chunk-pair pieces with
            # block-0's first slice injected after the first piece, so the
            # earliest matmuls (b-tiles 0-1 x chunks 0-1) start at ~5us
            # instead of waiting for half the table.
            ch_p = []
            for cpi in range(4):
                ch_t = cres.tile([P, 2, K], DT16, tag=f"chp{cpi}")
                ch_p.append(ch_t)
            nc.sync.dma_start(ch_p[0], ch_r[:, 0:2, :])
            x0_s = x0pool.tile([P, FCH, 2 * P], DT16)       # b-tiles 0-1
            nc.sync.dma_start(x0_s, xh_r[:, :, 0:2 * P])
            for cpi in range(1, 4):
                nc.sync.dma_start(ch_p[cpi], ch_r[:, 2 * cpi:2 * cpi + 2, :])

            x0_c = x0pool.tile([P, FCH, 2 * P], DT16)       # b-tiles 2-3
            nc.sync.dma_start(x0_c, xh_r[:, :, 2 * P:4 * P])

            # bias broadcast: c2s already holds 2^12 * ||c||^2 / 2. After
            # x0_c: the PE needs b-tile 2's data right when it would land
            # behind this 512KB transfer, while the first DVE subtract has
            # ~2us of pipeline slack to absorb the later bias arrival.
            c2b = cres.tile([P, K], mybir.dt.float32)
            nc.sync.dma_start(c2b, c2s[None, :].to_broadcast([P, K]))

            x0_d = x0pool.tile([P, FCH, 4 * P], DT16)       # b-tiles 4-7
            nc.sync.dma_start(x0_d, xh_r[:, :, 4 * P:BBLK])

            def ch_sl(f, h):
                return ch_p[f // 2][:, f % 2, h * NH:(h + 1) * NH]

            def load_blk(blk):
                t = xpool.tile([P, FCH, BBLK], DT16, tag="xblk")
                nc.sync.dma_start(
                    t, xh_r[:, :, blk * BBLK:(blk + 1) * BBLK])
                return t

            blk1 = load_blk(1)

            def epilogue(t, ps):
                a_sb = work.tile([P, K], mybir.dt.float32, tag="a")
                nc.vector.tensor_sub(a_sb, ps, c2b)
                # fused copy + row-max: out=a_sb+0.0 (exact identity, so
                # max_index's equality scan against a_sb still matches),
                # accum = row max. ~350ns cheaper than InstMax: the
                # all-SBUF tensor_scalar runs in the DVE 2x mode.
                a2 = work.tile([P, K], mybir.dt.float32, tag="a2")
                nc.vector.tensor_scalar(
                    out=a2, in0=a_sb, scalar1=0.0, scalar2=None,
                    op0=mybir.AluOpType.add, op1=mybir.AluOpType.max,
                    accum_out=mx[:, 0:1])
                ix = work.tile([P, 8], mybir.dt.uint32, tag="ix")
                nc.vector.max_index(ix, mx, a_sb)
                nc.sync.dma_start(out[t], ix[:, 0:1])

            for blk in range(nblk):
                if blk == 1:
                    xb = blk1
                elif blk >= 2:
                    xb = load_blk(blk)
                for i in range(TPB):
                    t = blk * TPB + i
                    if blk == 0:
                        if i < 2:
                            wt, j = x0_s, i
                        elif i < 4:
                            wt, j = x0_c, i - 2
                        else:
                            wt, j = x0_d, i - 4
                    else:
                        wt, j = xb, i
                    ps = psp.tile([P, K], mybir.dt.float32, tag="ps")
                    for f in range(FCH):
                        wh = wt[:, f, j * P:(j + 1) * P]
                        nc.tensor.matmul(ps[:, 0:NH], wh, ch_sl(f, 0),
                                         start=(f == 0), stop=(f == FCH - 1))
                        nc.tensor.matmul(ps[:, NH:K], wh, ch_sl(f, 1),
                                         start=(f == 0), stop=(f == FCH - 1))
                    epilogue(t, ps)
    nc.finalize()
    return nc


def _get_nc(bl):
    if bl not in _NC_CACHE:
        _NC_CACHE[bl] = _build(bl)
    return _NC_CACHE[bl]


def _prep(X, C):
    """Host-side operand prep: fp16 encodings + scaled bias.

    C is encoded centered (C - 0.5): the induced extra score term
    0.5*sum_i(x_i) is constant per row so the argmax is unchanged, while
    halving |C| roughly halves fp16 quantization error (27 vs 35 argmax
    flips on the reference inputs).
    """
    xh16 = (X * np.float32(S)).astype(np.float16)    # fp16(2^12 * X)
    ch16 = (C - np.float32(0.5)).astype(np.float16)
    xh_t = np.ascontiguousarray(xh16.T)
    ch_t = np.ascontiguousarray(ch16.T)
    # with centered scores s'_k = x@(c_k - 0.5), the objective
    # x@c_k - ||c_k||^2/2 equals s'_k - ||c_k||^2/2 plus the row-constant
    # x@0.5 which cannot change the argmax, so the bias is unchanged.
    c2s = (np.float64(S) * 0.5 * np.sum(C.astype(np.float64) ** 2, axis=1)
           ).astype(np.float32)
    return xh_t, ch_t, c2s


def kernel(X, centroids):
    X = np.ascontiguousarray(np.asarray(X, dtype=np.float32))
    C = np.ascontiguousarray(np.asarray(centroids, dtype=np.float32))
    assert X.shape == (B, F) and C.shape == (K, F)

    xh_t, ch_t, c2s = _prep(X, C)

    nc = _get_nc(BL)
    in_maps = []
    for c in range(NCORES):
        sl = slice(c * BL, (c + 1) * BL)
        in_maps.append({
            "xh": np.ascontiguousarray(xh_t[:, sl]),
            "ch": ch_t,
            "c2s": c2s,
        })
    res = run_bass_kernel_spmd(nc, in_maps, core_ids=list(range(NCORES)))
    out = np.concatenate([r["out"].reshape(-1) for r in res.results])
    return out.astype(np.int32)
